# revision 20
# baseline (speedup 1.0000x reference)
"""Trainium2 Bass kernel for nn_MultiHeadAttention_73504070303932.

Multi-head causal attention with a learned per-head key scale on a shared
K=V projection:

    q  = (x @ w_q) / sqrt(d)          reshaped to (b, h, L, v)
    kv = x @ w_kv                     reshaped to (b, h, L, v)
    k  = kv * (1 + k_weights)
    y  = softmax(causal(q k^T)) @ kv
    out = y @ w_o

Shapes: x [4, 1024, 1024], w_q/w_kv/w_o [1024, 1024],
k_weights [1, 16, 1, 64]; h=16 heads of v=64.

Sharding (8 NeuronCores): data-parallel over batch (4) x tensor-parallel
over head halves (2). Core c handles batch c//2 and heads (c%2)*8..+8.
After attention, each pair of cores AllGathers y^T and computes a
column-slice of the output projection, so the final output needs no
reduction.

On-core layout: everything is kept "transposed" (feature dim on
partitions):
  - projections compute q'^T [hv, L] and kv^T [hv, L] with the natural
    (untransposed) weight tiles as the stationary operand and x^T (host
    pre-transposed) as the moving operand; (1 + k_weights)/sqrt(d) is
    folded into q'.
  - kv in natural orientation [L, hv] (needed as the AV stationary
    operand) is produced by a DRAM round trip: kv^T is written out once
    and transpose-gathered back per j-tile, which is far cheaper than a
    third projection.
  - causal attention per head over j-tiles of 128 and i-blocks of 512:
    L^T[j, i] = kv^T_h(j)^T q'^T_h(i); exp on ACT over 4-j-tile groups
    (no max subtraction -- logits are O(1) by construction);
    diagonal-straddling groups are masked post-exp with a single 0/1
    mask; the AV matmul's stationary operand is kv natural with a 65th
    column of ones appended, so psum row 64 accumulates the softmax
    denominator for free; normalization multiplies by a K=2 broadcast
    matmul that serves both heads of a tile pair at once.
  - all tensor-engine operands are float32r (hw rounds to ~12 mantissa
    bits; measured 1.4e-4 rel err over K=1024; full PE rate at N>=256).
  - ops are batched into as few, as large instructions as possible
    (multi-bank psum tiles, single merged DMAs per tensor): this target
    executes instructions serially at a flat ~40-110 us each regardless
    of engine or data size, so wall time tracks total instruction count
    (~635/iteration here).

Measured on the target: relative error 2.7e-4 vs the fp32 jax reference;
~28-40 ms per iteration (machine-load dependent), vs ~120 ms for the
naive 1240-instruction version of the same algorithm.
"""

import math

import numpy as np

import concourse.bass as bass
import concourse.mybir as mybir
import concourse.tile as tile
from concourse import bacc
from concourse.bass_utils import run_bass_kernel_spmd

F32 = mybir.dt.float32
F32R = mybir.dt.float32r

N_CORES = 8
B, SEQ, D = 4, 1024, 1024
H, V = 16, 64
HL = 8          # heads per core
HV = HL * V     # 512 local feature dim
P = 128         # partitions
IB = 512        # i-block (query) width in the attention loop
NI = SEQ // IB  # 2 i-blocks
ND = D // P     # 8 d-tiles
NS = SEQ // P   # 8 seq j-tiles
NHV = HV // P   # 4 local hv tiles
KVC = V + 1     # kv columns per head incl the ones column
GW = 4 * IB     # exp/mask group width: 4 j-tiles' logits side by side


def build_program(n_iters: int = 1):
    """Build the SPMD program (same for all 8 cores). Returns compiled nc.

    n_iters > 1 replicates the whole body for on-device timing runs.
    """
    nc = bacc.Bacc(trn_type="TRN2", target_bir_lowering=False, debug=False,
                   num_devices=N_CORES)

    xT = nc.dram_tensor("xT", [D, SEQ], F32R, kind="ExternalInput").ap()
    wq = nc.dram_tensor("wq", [D, HV], F32R, kind="ExternalInput").ap()
    wkv = nc.dram_tensor("wkv", [D, HV], F32R, kind="ExternalInput").ap()
    wo = nc.dram_tensor("wo", [D, HV], F32R, kind="ExternalInput").ap()
    kws = nc.dram_tensor("kws", [P, NHV], F32, kind="ExternalInput").ap()
    # 0/1 causal mask for a diagonal-straddling group of 4 j-tiles
    maskq = nc.dram_tensor("maskq", [P, GW], F32, kind="ExternalInput").ap()
    out = nc.dram_tensor("out", [SEQ, HV], F32, kind="ExternalOutput").ap()

    Exp = mybir.ActivationFunctionType.Exp

    with tile.TileContext(nc) as tc:
        with (
            tc.tile_pool(name="consts", bufs=1) as consts,
            tc.tile_pool(name="dram", bufs=1, space="DRAM") as dram,
        ):
            kws_sb = consts.tile([P, NHV], F32)
            maskq_sb = consts.tile([P, GW], F32)
            ones8_f = consts.tile([P, NS * HL], F32)
            ones64_f = consts.tile([1, V], F32)
            ones64_r = consts.tile([1, V], F32R)
            nc.sync.dma_start(kws_sb[:], kws[:])
            nc.sync.dma_start(maskq_sb[:], maskq[:])
            nc.vector.memset(ones8_f[:], 1.0)
            nc.vector.memset(ones64_f[:], 1.0)
            nc.vector.tensor_copy(ones64_r[:], ones64_f[:])

            kv_dram = dram.tile([HV, SEQ], F32R)
            y_loc = dram.tile([HV, SEQ], F32R)
            y_all = dram.tile([2 * HV, SEQ], F32R)

            for it in range(n_iters):
                _one_iter(nc, tc, it, xT, wq, wkv, wo, out,
                          kws_sb, maskq_sb, ones8_f, ones64_r,
                          kv_dram, y_loc, y_all, Exp)

    nc.compile()
    return nc


def _one_iter(nc, tc, it, xT, wq, wkv, wo, out,
              kws_sb, maskq_sb, ones8_f, ones64_r,
              kv_dram, y_loc, y_all, Exp):
    with (
        tc.tile_pool(name=f"qkv{it}", bufs=1) as qkv,
        tc.tile_pool(name=f"wop{it}", bufs=1) as wop,
        tc.tile_pool(name=f"ytp{it}", bufs=1) as ytp,
    ):
        # persistent on-core tensors, each a single merged tile
        qT = qkv.tile([P, NHV * SEQ], F32R, tag="qT", name="qT")
        kvT = qkv.tile([P, NHV * SEQ], F32R, tag="kvT", name="kvT")
        kvn = qkv.tile([P, NS * HL * KVC], F32R, tag="kvn", name="kvn")
        wo_sb = wop.tile([P, ND * HV], F32R, tag="wos", name="wos")
        yT = ytp.tile([P, NHV * SEQ], F32R, tag="yTt", name="yTt")
        nc.sync.dma_start(wo_sb[:].rearrange("p (k n) -> p k n", n=HV),
                          wo.rearrange("(k p) n -> p k n", p=P))

        # ---- load x^T + weights, project q'^T and kv^T ----
        with (
            tc.tile_pool(name=f"xw{it}", bufs=1) as xw,
            tc.tile_pool(name=f"mmps{it}", bufs=3, space="PSUM") as mmps,
        ):
            xT_sb = xw.tile([P, ND * SEQ], F32R, tag="xTs", name="xTs")
            wq_sb = xw.tile([P, ND * HV], F32R, tag="wqs", name="wqs")
            wkv_sb = xw.tile([P, ND * HV], F32R, tag="wkvs", name="wkvs")
            kvst = xw.tile([P, NS * HV], F32R, tag="kvst", name="kvst")
            nc.sync.dma_start(xT_sb[:].rearrange("p (k s) -> p k s", s=SEQ),
                              xT.rearrange("(k p) s -> p k s", p=P))
            nc.sync.dma_start(wq_sb[:].rearrange("p (k n) -> p k n", n=HV),
                              wq.rearrange("(k p) n -> p k n", p=P))
            nc.sync.dma_start(wkv_sb[:].rearrange("p (k n) -> p k n", n=HV),
                              wkv.rearrange("(k p) n -> p k n", p=P))

            # q^T / kv^T: [hv-tile m, seq] = sum_k w[:, m]^T @ x^T
            for m in range(NHV):
                ps_q = mmps.tile([P, SEQ], F32, tag="mm", name="ps_q")
                ps_k = mmps.tile([P, SEQ], F32, tag="mm", name="ps_k")
                for n in range(SEQ // 512):
                    for k in range(ND):
                        wq_k = wq_sb[:, k * HV + m * P:k * HV + (m + 1) * P]
                        wkv_k = wkv_sb[:, k * HV + m * P:k * HV + (m + 1) * P]
                        x_k = xT_sb[:, k * SEQ + n * 512:k * SEQ + (n + 1) * 512]
                        nc.tensor.matmul(ps_q[:, n * 512:(n + 1) * 512],
                                         wq_k, x_k,
                                         start=(k == 0), stop=(k == ND - 1))
                        nc.tensor.matmul(ps_k[:, n * 512:(n + 1) * 512],
                                         wkv_k, x_k,
                                         start=(k == 0), stop=(k == ND - 1))
                nc.vector.tensor_scalar_mul(
                    qT[:, m * SEQ:(m + 1) * SEQ], ps_q[:], kws_sb[:, m:m + 1])
                nc.vector.tensor_copy(kvT[:, m * SEQ:(m + 1) * SEQ], ps_k[:])
            nc.sync.dma_start(kv_dram.rearrange("(m p) s -> p m s", p=P),
                              kvT[:].rearrange("p (m s) -> p m s", s=SEQ))
            for t in range(NS):
                nc.sync.dma_start(
                    kvst[:, t * HV:(t + 1) * HV],
                    bass.AP(kv_dram.tensor, kv_dram.offset + t * P,
                            [[1, P], [SEQ, HV]]))
            nc.vector.tensor_copy(
                kvn[:].rearrange("p (t h c) -> p t h c", h=HL, c=KVC)[:, :, :, 0:V],
                kvst[:].rearrange("p (t h c) -> p t h c", h=HL, c=V))
            # ones columns for every (j-tile, head) in one strided write
            nc.vector.tensor_copy(
                kvn[:].rearrange("p (t c) -> p t c", c=KVC)[:, :, V:V + 1],
                ones8_f[:].rearrange("p (t o) -> p t o", o=1))

        # kv natural via DRAM round trip: write kv^T once, transpose-gather
        # one contiguous j-tile per DMA, then scatter into the
        # 65-column-per-head layout in one strided copy

        # ---- causal attention, head-pair by head-pair ----
        with (
            tc.tile_pool(name=f"st{it}", bufs=5) as stp,
            tc.tile_pool(name=f"sd{it}", bufs=2) as sdp,
            tc.tile_pool(name=f"rs{it}", bufs=4) as rsp,
            tc.tile_pool(name=f"ltps{it}", bufs=1, space="PSUM") as ltps,
            tc.tile_pool(name=f"yps{it}", bufs=3, space="PSUM") as yps,
            tc.tile_pool(name=f"bps{it}", bufs=1, space="PSUM") as bps,
        ):
            for u in range(HL // 2):         # head pair (2u, 2u+1), tile u
                for i in range(NI):
                    nj = (i + 1) * IB // P   # causal j-tiles
                    ng = nj // 4             # exp groups of 4 j-tiles
                    ps_ys = []
                    sts = {}
                    for hh in range(2):
                        r0 = hh * V
                        # QK in groups of 4 j-tiles -> one 4-bank psum
                        for g in range(ng):
                            ps_l = ltps.tile([P, GW], F32, tag="lt",
                                             name="ps_l")
                            for jo in range(4):
                                j = 4 * g + jo
                                nc.tensor.matmul(
                                    ps_l[:, jo * IB:(jo + 1) * IB],
                                    kvT[r0:r0 + V,
                                        u * SEQ + j * P:u * SEQ + (j + 1) * P],
                                    qT[r0:r0 + V,
                                       u * SEQ + i * IB:u * SEQ + (i + 1) * IB],
                                    start=True, stop=True)
                            st = stp.tile([P, GW], F32R, tag="st", name="st")
                            if g == ng - 1:  # diagonal-straddling group
                                sd = sdp.tile([P, GW], F32, tag="sd", name="sd")
                                nc.scalar.activation(sd[:], ps_l[:], Exp)
                                nc.vector.tensor_tensor(
                                    st[:], sd[:], maskq_sb[:],
                                    mybir.AluOpType.mult)
                            else:
                                nc.scalar.activation(st[:], ps_l[:], Exp)
                            sts[(hh, g)] = st
                        # AV with ones row: psum rows 0..63 = y, row 64 = sum
                        ps_y = yps.tile([P, IB], F32, tag="y", name="ps_y")
                        h = 2 * u + hh
                        for j in range(nj):
                            nc.tensor.matmul(
                                ps_y[0:V + 1, :],
                                kvn[:, (j * HL + h) * KVC:
                                    (j * HL + h + 1) * KVC],
                                sts[(hh, j // 4)][:, (j % 4) * IB:
                                                  (j % 4 + 1) * IB],
                                start=(j == 0), stop=(j == nj - 1))
                        ps_ys.append(ps_y)
                    # normalization per head: K=1 broadcast matmul of 1/sum
                    for hh in range(2):
                        rs_r = rsp.tile([1, IB], F32R, tag="rsr", name="rs_r")
                        with nc.allow_low_precision(
                                reason="denom rounds to f32r"):
                            nc.vector.reciprocal(rs_r[:],
                                                 ps_ys[hh][V:V + 1, :])
                        ps_b = bps.tile([V, IB], F32, tag="b", name="ps_b")
                        nc.tensor.matmul(ps_b[:], ones64_r[:], rs_r[:],
                                         start=True, stop=True)
                        sb_b = rsp.tile([V, IB], F32, tag="sbb", name="sb_b")
                        nc.vector.tensor_copy(sb_b[:], ps_b[:])
                        nc.vector.tensor_tensor(
                            yT[hh * V:(hh + 1) * V,
                               u * SEQ + i * IB:u * SEQ + (i + 1) * IB],
                            ps_ys[hh][0:V, :], sb_b[:],
                            mybir.AluOpType.mult)

        # ---- exchange y^T halves within the batch pair ----
        nc.sync.dma_start(y_loc.rearrange("(m p) s -> p m s", p=P),
                          yT[:].rearrange("p (m s) -> p m s", s=SEQ))
        nc.gpsimd.collective_compute(
            "AllGather",
            mybir.AluOpType.bypass,
            replica_groups=[[0, 1], [2, 3], [4, 5], [6, 7]],
            ins=[y_loc.opt()],
            outs=[y_all.opt()],
        )
        with (
            tc.tile_pool(name=f"yf{it}", bufs=1) as yfp,
            tc.tile_pool(name=f"os{it}", bufs=1) as osp,
            tc.tile_pool(name=f"ops{it}", bufs=3, space="PSUM") as ops,
        ):
            yF = yfp.tile([P, 2 * NHV * SEQ], F32R, tag="yF", name="yF")
            nc.sync.dma_start(yF[:].rearrange("p (g s) -> p g s", s=SEQ),
                              y_all.rearrange("(g p) s -> p g s", p=P))

            # ---- output projection: out[:, my cols] = y^T.T @ wo_cols ----
            o_sb = osp.tile([P, NS * HV], F32, tag="osb", name="osb")
            for m2 in range(NS // 2):
                ps = ops.tile([P, 2 * HV], F32, tag="om", name="ps_o")
                for half in range(2):
                    m = 2 * m2 + half
                    for g in range(2 * NHV):
                        nc.tensor.matmul(
                            ps[:, half * HV:(half + 1) * HV],
                            yF[:, g * SEQ + m * P:g * SEQ + (m + 1) * P],
                            wo_sb[:, g * HV:(g + 1) * HV],
                            start=(g == 0), stop=(g == 2 * NHV - 1))
                nc.vector.tensor_copy(
                    o_sb[:, m2 * 2 * HV:(m2 + 1) * 2 * HV], ps[:])
            nc.sync.dma_start(out.rearrange("(m p) n -> p m n", p=P),
                              o_sb[:].rearrange("p (m n) -> p m n", n=HV))


def shard_inputs(x, w_q, w_kv, w_o, k_weights):
    """Full inputs -> list of 8 per-core input dicts."""
    scale = 1.0 / math.sqrt(D)
    jj = np.arange(P)[:, None]
    ii = np.arange(IB)[None, :]
    maskq = np.concatenate(
        [(ii >= jj + o * P).astype(np.float32) for o in range(4)], axis=1)
    in_maps = []
    for c in range(N_CORES):
        b, half = c // 2, c % 2
        cols = slice(half * HV, (half + 1) * HV)
        kw = (1.0 + k_weights[0, half * HL:(half + 1) * HL, 0, :]) * scale
        kws = np.ascontiguousarray(
            kw.reshape(HV).reshape(NHV, P).T).astype(np.float32)
        in_maps.append({
            "xT": np.ascontiguousarray(x[b].T).astype(np.float32),
            "wq": np.ascontiguousarray(w_q[:, cols]).astype(np.float32),
            "wkv": np.ascontiguousarray(w_kv[:, cols]).astype(np.float32),
            "wo": np.ascontiguousarray(w_o[:, cols]).astype(np.float32),
            "kws": kws,
            "maskq": maskq,
        })
    return in_maps


_CACHED_NC = None


def kernel(x, w_q, w_kv, w_o, k_weights):
    """Full (unsharded) inputs -> full [4, 1024, 1024] output."""
    global _CACHED_NC
    if _CACHED_NC is None:
        _CACHED_NC = build_program()
    nc = _CACHED_NC
    in_maps = shard_inputs(np.asarray(x, dtype=np.float32),
                           np.asarray(w_q, dtype=np.float32),
                           np.asarray(w_kv, dtype=np.float32),
                           np.asarray(w_o, dtype=np.float32),
                           np.asarray(k_weights, dtype=np.float32))
    res = run_bass_kernel_spmd(nc, in_maps, list(range(N_CORES)))
    outs = [
        np.concatenate([res.results[2 * b]["out"], res.results[2 * b + 1]["out"]],
                       axis=1)
        for b in range(B)
    ]
    return np.stack(outs, axis=0)


# revision 23
# speedup vs baseline: 1.3103x; 1.3103x over previous
"""Trainium2 Bass kernel for nn_MultiHeadAttention_73504070303932.

Multi-head causal attention with a learned per-head key scale on a shared
K=V projection:

    q  = (x @ w_q) / sqrt(d)          reshaped to (b, h, L, v)
    kv = x @ w_kv                     reshaped to (b, h, L, v)
    k  = kv * (1 + k_weights)
    y  = softmax(causal(q k^T)) @ kv
    out = y @ w_o

Shapes: x [4, 1024, 1024], w_q/w_kv/w_o [1024, 1024],
k_weights [1, 16, 1, 64]; h=16 heads of v=64.

Sharding (8 NeuronCores): data-parallel over batch (4) x tensor-parallel
over head halves (2). Core c handles batch c//2 and heads (c%2)*8..+8.
After attention, each pair of cores AllGathers y^T and computes a
column-slice of the output projection, so the final output needs no
reduction.

On-core layout: everything is kept "transposed" (feature dim on
partitions):
  - projections compute q'^T [hv, L] and kv^T [hv, L] with the natural
    (untransposed) weight tiles as the stationary operand and x^T (host
    pre-transposed) as the moving operand; (1 + k_weights)/sqrt(d) is
    folded into q'.
  - kv in natural orientation [L, hv] (needed as the AV stationary
    operand) is produced by a DRAM round trip: kv^T is written out once
    and transpose-gathered back per j-tile, which is far cheaper than a
    third projection.
  - causal attention per head over j-tiles of 128 and i-blocks of 512:
    L^T[j, i] = kv^T_h(j)^T q'^T_h(i); exp on ACT over 4-j-tile groups
    (no max subtraction -- logits are O(1) by construction);
    diagonal-straddling groups are masked post-exp with a single 0/1
    mask; the AV matmul's stationary operand is kv natural with a 65th
    column of ones appended, so psum row 64 accumulates the softmax
    denominator for free; normalization multiplies by a K=2 broadcast
    matmul that serves both heads of a tile pair at once.
  - all tensor-engine operands are float32r (hw rounds to ~12 mantissa
    bits; measured 1.4e-4 rel err over K=1024; full PE rate at N>=256).
  - ops are batched into as few, as large instructions as possible
    (multi-bank psum tiles, single merged DMAs per tensor): this target
    executes instructions serially at a flat ~40-110 us each regardless
    of engine or data size, so wall time tracks total instruction count
    (~635/iteration here).

Measured on the target: relative error 2.7e-4 vs the fp32 jax reference;
~28-40 ms per iteration (machine-load dependent), vs ~120 ms for the
naive 1240-instruction version of the same algorithm.
"""

import math

import numpy as np

import concourse.bass as bass
import concourse.mybir as mybir
import concourse.tile as tile
from concourse import bacc
from concourse.bass_utils import run_bass_kernel_spmd

F32 = mybir.dt.float32
F32R = mybir.dt.float32r

N_CORES = 8
B, SEQ, D = 4, 1024, 1024
H, V = 16, 64
HL = 8          # heads per core
HV = HL * V     # 512 local feature dim
P = 128         # partitions
IB = 512        # i-block (query) width in the attention loop
NI = SEQ // IB  # 2 i-blocks
ND = D // P     # 8 d-tiles
NS = SEQ // P   # 8 seq j-tiles
NHV = HV // P   # 4 local hv tiles
KVC = V + 1     # kv columns per head incl the ones column
GW = 4 * IB     # exp/mask group width: 4 j-tiles' logits side by side


def build_program(n_iters: int = 1):
    """Build the SPMD program (same for all 8 cores). Returns compiled nc.

    n_iters > 1 replicates the whole body for on-device timing runs.
    """
    nc = bacc.Bacc(trn_type="TRN2", target_bir_lowering=False, debug=False,
                   num_devices=N_CORES)

    xT = nc.dram_tensor("xT", [D, SEQ], F32R, kind="ExternalInput").ap()
    wq = nc.dram_tensor("wq", [D, HV], F32R, kind="ExternalInput").ap()
    wkv = nc.dram_tensor("wkv", [D, HV], F32R, kind="ExternalInput").ap()
    wo = nc.dram_tensor("wo", [D, HV], F32R, kind="ExternalInput").ap()
    # 0/1 causal mask for a diagonal-straddling group of 4 j-tiles
    maskq = nc.dram_tensor("maskq", [P, GW], F32, kind="ExternalInput").ap()
    out = nc.dram_tensor("out", [SEQ, HV], F32, kind="ExternalOutput").ap()

    Exp = mybir.ActivationFunctionType.Exp

    with tile.TileContext(nc) as tc:
        with (
            tc.tile_pool(name="consts", bufs=1) as consts,
            tc.tile_pool(name="dram", bufs=1, space="DRAM") as dram,
        ):
            maskq_sb = consts.tile([P, GW], F32)
            ones8_f = consts.tile([P, NS * HL], F32)
            ones64_f = consts.tile([1, V], F32)
            ones64_r = consts.tile([1, V], F32R)
            nc.sync.dma_start(maskq_sb[:], maskq[:])
            nc.vector.memset(ones8_f[:], 1.0)
            nc.vector.memset(ones64_f[:], 1.0)
            nc.vector.tensor_copy(ones64_r[:], ones64_f[:])

            kv_dram = dram.tile([HV, SEQ], F32R)
            y_loc = dram.tile([HV, SEQ], F32R)
            y_all = dram.tile([2 * HV, SEQ], F32R)

            for it in range(n_iters):
                _one_iter(nc, tc, it, xT, wq, wkv, wo, out,
                          maskq_sb, ones8_f, ones64_r,
                          kv_dram, y_loc, y_all, Exp)

    nc.compile()
    return nc


def _one_iter(nc, tc, it, xT, wq, wkv, wo, out,
              maskq_sb, ones8_f, ones64_r,
              kv_dram, y_loc, y_all, Exp):
    with (
        tc.tile_pool(name=f"qkv{it}", bufs=1) as qkv,
        tc.tile_pool(name=f"wop{it}", bufs=1) as wop,
        tc.tile_pool(name=f"ytp{it}", bufs=1) as ytp,
    ):
        # persistent on-core tensors, each a single merged tile
        qT = qkv.tile([P, NHV * SEQ], F32R, tag="qT", name="qT")
        kvT = qkv.tile([P, NHV * SEQ], F32R, tag="kvT", name="kvT")
        kvn = qkv.tile([P, NS * HL * KVC], F32R, tag="kvn", name="kvn")
        wo_sb = wop.tile([P, ND * HV], F32R, tag="wos", name="wos")
        yT = ytp.tile([P, NHV * SEQ], F32R, tag="yTt", name="yTt")
        nc.sync.dma_start(wo_sb[:].rearrange("p (k n) -> p k n", n=HV),
                          wo.rearrange("(k p) n -> p k n", p=P))

        # ---- load x^T + weights, project q'^T and kv^T ----
        with (
            tc.tile_pool(name=f"xw{it}", bufs=1) as xw,
            tc.tile_pool(name=f"mmps{it}", bufs=2, space="PSUM") as mmps,
        ):
            xT_sb = xw.tile([P, ND * SEQ], F32R, tag="xTs", name="xTs")
            wq_sb = xw.tile([P, ND * HV], F32R, tag="wqs", name="wqs")
            wkv_sb = xw.tile([P, ND * HV], F32R, tag="wkvs", name="wkvs")
            kvst = xw.tile([P, NS * HV], F32R, tag="kvst", name="kvst")
            nc.sync.dma_start(xT_sb[:].rearrange("p (k s) -> p k s", s=SEQ),
                              xT.rearrange("(k p) s -> p k s", p=P))
            nc.sync.dma_start(wq_sb[:].rearrange("p (k n) -> p k n", n=HV),
                              wq.rearrange("(k p) n -> p k n", p=P))
            nc.sync.dma_start(wkv_sb[:].rearrange("p (k n) -> p k n", n=HV),
                              wkv.rearrange("(k p) n -> p k n", p=P))

            # q^T / kv^T: [hv-tile m, seq] = sum_k w[:, m]^T @ x^T;
            # (1+k_weights)/sqrt(d) is pre-folded into wq's columns on host
            for mp in range(NHV // 2):
                ps_q = mmps.tile([P, 2 * SEQ], F32, tag="mm", name="ps_q")
                ps_k = mmps.tile([P, 2 * SEQ], F32, tag="mm", name="ps_k")
                for mo in range(2):
                    m = 2 * mp + mo
                    for n in range(SEQ // 512):
                        c0 = mo * SEQ + n * 512
                        for k in range(ND):
                            wq_k = wq_sb[:, k * HV + m * P:k * HV + (m + 1) * P]
                            wkv_k = wkv_sb[:, k * HV + m * P:k * HV + (m + 1) * P]
                            x_k = xT_sb[:, k * SEQ + n * 512:k * SEQ + (n + 1) * 512]
                            nc.tensor.matmul(ps_q[:, c0:c0 + 512], wq_k, x_k,
                                             start=(k == 0), stop=(k == ND - 1))
                            nc.tensor.matmul(ps_k[:, c0:c0 + 512], wkv_k, x_k,
                                             start=(k == 0), stop=(k == ND - 1))
                nc.vector.tensor_copy(
                    qT[:, 2 * mp * SEQ:2 * (mp + 1) * SEQ], ps_q[:])
                nc.vector.tensor_copy(
                    kvT[:, 2 * mp * SEQ:2 * (mp + 1) * SEQ], ps_k[:])
            nc.sync.dma_start(kv_dram.rearrange("(m p) s -> p m s", p=P),
                              kvT[:].rearrange("p (m s) -> p m s", s=SEQ))
            for t in range(NS):
                nc.sync.dma_start(
                    kvst[:, t * HV:(t + 1) * HV],
                    bass.AP(kv_dram.tensor, kv_dram.offset + t * P,
                            [[1, P], [SEQ, HV]]))
            nc.vector.tensor_copy(
                kvn[:].rearrange("p (t h c) -> p t h c", h=HL, c=KVC)[:, :, :, 0:V],
                kvst[:].rearrange("p (t h c) -> p t h c", h=HL, c=V))
            # ones columns for every (j-tile, head) in one strided write
            nc.vector.tensor_copy(
                kvn[:].rearrange("p (t c) -> p t c", c=KVC)[:, :, V:V + 1],
                ones8_f[:].rearrange("p (t o) -> p t o", o=1))

        # kv natural via DRAM round trip: write kv^T once, transpose-gather
        # one contiguous j-tile per DMA, then scatter into the
        # 65-column-per-head layout in one strided copy

        # ---- causal attention, head by head ----
        with (
            tc.tile_pool(name=f"st{it}", bufs=5) as stp,
            tc.tile_pool(name=f"sd{it}", bufs=2) as sdp,
            tc.tile_pool(name=f"rs{it}", bufs=4) as rsp,
            tc.tile_pool(name=f"ltps{it}", bufs=1, space="PSUM") as ltps,
            tc.tile_pool(name=f"yps{it}", bufs=1, space="PSUM") as yps,
            tc.tile_pool(name=f"bps{it}", bufs=1, space="PSUM") as bps,
        ):
            for u in range(HL // 2):         # head pair (2u, 2u+1), tile u
                for hh in range(2):
                    h, r0 = 2 * u + hh, hh * V
                    # both i-blocks of this head share one 2-bank AV psum
                    ps_y = yps.tile([P, SEQ // NI * NI], F32, tag="y",
                                    name="ps_y", padded_shape=[P, NI * IB])
                    sts = {}
                    for i in range(NI):
                        nj = (i + 1) * IB // P   # causal j-tiles
                        ng = nj // 4             # exp groups of 4 j-tiles
                        for g in range(ng):
                            ps_l = ltps.tile([P, GW], F32, tag="lt",
                                             name="ps_l")
                            for jo in range(4):
                                j = 4 * g + jo
                                nc.tensor.matmul(
                                    ps_l[:, jo * IB:(jo + 1) * IB],
                                    kvT[r0:r0 + V,
                                        u * SEQ + j * P:u * SEQ + (j + 1) * P],
                                    qT[r0:r0 + V,
                                       u * SEQ + i * IB:u * SEQ + (i + 1) * IB],
                                    start=True, stop=True)
                            st = stp.tile([P, GW], F32R, tag="st", name="st")
                            if g == ng - 1:  # diagonal-straddling group
                                sd = sdp.tile([P, GW], F32, tag="sd",
                                              name="sd")
                                nc.scalar.activation(sd[:], ps_l[:], Exp)
                                nc.vector.tensor_tensor(
                                    st[:], sd[:], maskq_sb[:],
                                    mybir.AluOpType.mult)
                            else:
                                nc.scalar.activation(st[:], ps_l[:], Exp)
                            sts[(i, g)] = st
                        for j in range(nj):
                            nc.tensor.matmul(
                                ps_y[0:V + 1, i * IB:(i + 1) * IB],
                                kvn[:, (j * HL + h) * KVC:
                                    (j * HL + h + 1) * KVC],
                                sts[(i, j // 4)][:, (j % 4) * IB:
                                                 (j % 4 + 1) * IB],
                                start=(j == 0), stop=(j == nj - 1))
                    # normalize the whole head row at once
                    rs_r = rsp.tile([1, NI * IB], F32R, tag="rsr", name="rs_r")
                    with nc.allow_low_precision(reason="denom rounds to f32r"):
                        nc.vector.reciprocal(rs_r[:], ps_y[V:V + 1, :])
                    ps_b = bps.tile([V, NI * IB], F32, tag="b", name="ps_b")
                    for i in range(NI):
                        nc.tensor.matmul(ps_b[:, i * IB:(i + 1) * IB],
                                         ones64_r[:],
                                         rs_r[:, i * IB:(i + 1) * IB],
                                         start=True, stop=True)
                    sb_b = rsp.tile([V, NI * IB], F32, tag="sbb", name="sb_b")
                    nc.vector.tensor_copy(sb_b[:], ps_b[:])
                    nc.vector.tensor_tensor(
                        yT[r0:r0 + V, u * SEQ:(u + 1) * SEQ],
                        ps_y[0:V, :], sb_b[:], mybir.AluOpType.mult)

        # ---- exchange y^T halves within the batch pair ----
        nc.sync.dma_start(y_loc.rearrange("(m p) s -> p m s", p=P),
                          yT[:].rearrange("p (m s) -> p m s", s=SEQ))
        nc.gpsimd.collective_compute(
            "AllGather",
            mybir.AluOpType.bypass,
            replica_groups=[[0, 1], [2, 3], [4, 5], [6, 7]],
            ins=[y_loc.opt()],
            outs=[y_all.opt()],
        )
        with (
            tc.tile_pool(name=f"yf{it}", bufs=1) as yfp,
            tc.tile_pool(name=f"os{it}", bufs=1) as osp,
            tc.tile_pool(name=f"ops{it}", bufs=3, space="PSUM") as ops,
        ):
            yF = yfp.tile([P, 2 * NHV * SEQ], F32R, tag="yF", name="yF")
            nc.sync.dma_start(yF[:].rearrange("p (g s) -> p g s", s=SEQ),
                              y_all.rearrange("(g p) s -> p g s", p=P))

            # ---- output projection: out[:, my cols] = y^T.T @ wo_cols ----
            o_sb = osp.tile([P, NS * HV], F32, tag="osb", name="osb")
            for m2 in range(NS // 2):
                ps = ops.tile([P, 2 * HV], F32, tag="om", name="ps_o")
                for half in range(2):
                    m = 2 * m2 + half
                    for g in range(2 * NHV):
                        nc.tensor.matmul(
                            ps[:, half * HV:(half + 1) * HV],
                            yF[:, g * SEQ + m * P:g * SEQ + (m + 1) * P],
                            wo_sb[:, g * HV:(g + 1) * HV],
                            start=(g == 0), stop=(g == 2 * NHV - 1))
                nc.vector.tensor_copy(
                    o_sb[:, m2 * 2 * HV:(m2 + 1) * 2 * HV], ps[:])
            nc.sync.dma_start(out.rearrange("(m p) n -> p m n", p=P),
                              o_sb[:].rearrange("p (m n) -> p m n", n=HV))


def shard_inputs(x, w_q, w_kv, w_o, k_weights):
    """Full inputs -> list of 8 per-core input dicts."""
    scale = 1.0 / math.sqrt(D)
    jj = np.arange(P)[:, None]
    ii = np.arange(IB)[None, :]
    maskq = np.concatenate(
        [(ii >= jj + o * P).astype(np.float32) for o in range(4)], axis=1)
    in_maps = []
    for c in range(N_CORES):
        b, half = c // 2, c % 2
        cols = slice(half * HV, (half + 1) * HV)
        # fold (1 + k_weights)/sqrt(d) into wq's columns
        kw = (1.0 + k_weights[0, half * HL:(half + 1) * HL, 0, :]) * scale
        wq_scaled = w_q[:, cols].astype(np.float64) * kw.reshape(HV)[None, :]
        in_maps.append({
            "xT": np.ascontiguousarray(x[b].T).astype(np.float32),
            "wq": np.ascontiguousarray(wq_scaled).astype(np.float32),
            "wkv": np.ascontiguousarray(w_kv[:, cols]).astype(np.float32),
            "wo": np.ascontiguousarray(w_o[:, cols]).astype(np.float32),
            "maskq": maskq,
        })
    return in_maps


_CACHED_NC = None


def kernel(x, w_q, w_kv, w_o, k_weights):
    """Full (unsharded) inputs -> full [4, 1024, 1024] output."""
    global _CACHED_NC
    if _CACHED_NC is None:
        _CACHED_NC = build_program()
    nc = _CACHED_NC
    in_maps = shard_inputs(np.asarray(x, dtype=np.float32),
                           np.asarray(w_q, dtype=np.float32),
                           np.asarray(w_kv, dtype=np.float32),
                           np.asarray(w_o, dtype=np.float32),
                           np.asarray(k_weights, dtype=np.float32))
    res = run_bass_kernel_spmd(nc, in_maps, list(range(N_CORES)))
    outs = [
        np.concatenate([res.results[2 * b]["out"], res.results[2 * b + 1]["out"]],
                       axis=1)
        for b in range(B)
    ]
    return np.stack(outs, axis=0)
